# revision 2
# baseline (speedup 1.0000x reference)
"""Trainium2 Bass kernel for nn_Decoder_58514634440787 (histogram_binning).

Piecewise-linear decoder: y[b, s] = interp of (segment_x, segment_y) knots
evaluated at the uniform pixel grid t_s = (s+1)/S, S = 196608, B = 8.

Strategy: the output along the pixel axis is piecewise linear with at most
33 knots per batch.  Pixels are sharded across 8 cores (24576 each) and laid
out on-chip as [128 partitions = 8 batches x 16 rows, 1536 pixels].  Each
1536-pixel row intersects at most a couple of knots, so per row the host
ships tiny line parameters (slope/intercept + knot thresholds) and the
device evaluates:

    out = base_a[p] * t + base_b[p]                       (one tensor_scalar)
    patch left  half where t <  tL[p] with line (aL, bL)  (mask + line + copy_predicated)
    patch right half where t >= tR[p] with line (aR, bR)

The mask compares are the exact same f32 predicates (t >= x_n) the reference
uses, so segment selection is bit-identical; line evaluation a*t+b matches
the reference's ratio*(x_in-x)+y to ~3e-4 elementwise worst case.

Inputs are the full [8, 33] knot tensors; sharding/gather happens here.
"""

import numpy as np

S = 196608
B = 8
W = 1536              # pixels per partition row
RPB = 16              # rows per batch per core
P = 128               # partitions = B * RPB
NCORES = 8
PIX_PER_CORE = RPB * W  # 24576
NCHUNK = 4
CW = W // NCHUNK      # 384 columns per chunk
HALF = W // 2

_t_grid = None          # f32 [S] exact (s+1)/S
_t_tiles = None         # per-core [128, 1536] f32
_compiled = {}          # (n_left, n_right) -> nc


def _get_t():
    global _t_grid, _t_tiles
    if _t_grid is None:
        _t_grid = (np.arange(1, S + 1, dtype=np.float64) / S).astype(np.float32)
        tiles = []
        for c in range(NCORES):
            base = c * PIX_PER_CORE
            rows = _t_grid[base:base + PIX_PER_CORE].reshape(RPB, W)
            tiles.append(np.tile(rows, (B, 1)))  # [128, 1536]
        _t_tiles = tiles
    return _t_grid, _t_tiles


def _fix_x_order(sx, sy):
    """Running max of x along the segment axis, y carried from the position
    achieving the max (ties keep the later entry). Matches reference."""
    x = sx.copy()
    y = sy.copy()
    for b in range(sx.shape[0]):
        cx, cy = sx[b, 0], sy[b, 0]
        for i in range(sx.shape[1]):
            if sx[b, i] >= cx:
                cx, cy = sx[b, i], sy[b, i]
            x[b, i] = cx
            y[b, i] = cy
    return x, y


def _host_prep(segment_x, segment_y):
    """Returns (params_per_core, (n_left, n_right)).

    params_per_core: list of [128, 2 + 3*(n_left + n_right)] f32 arrays with
    columns [base_a, base_b, (tL_j, aL_j, bL_j)..., (tR_j, aR_j, bR_j)...].
    Left slots are ordered latest-breakpoint-first (applied in order, each
    narrower than the previous); right slots earliest-first.
    """
    t_grid, _ = _get_t()
    sx = np.asarray(segment_x, dtype=np.float32)
    sy = np.asarray(segment_y, dtype=np.float32)
    x, y = _fix_x_order(sx, sy)

    gaps = x[:, 1:] - x[:, :-1]
    div = np.where(gaps == 0.0, np.float32(0.0001), gaps).astype(np.float32)
    a = ((y[:, 1:] - y[:, :-1]) / div).astype(np.float32)          # [B, 32]
    b2 = (y[:, :-1].astype(np.float64)
          - a.astype(np.float64) * x[:, :-1].astype(np.float64)).astype(np.float32)

    # First pixel index s with t_s >= x_n, for binning knots n = 1..31.
    # searchsorted on the exact f32 grid == the reference's f32 compares.
    k = np.stack([np.searchsorted(t_grid, x[b, 1:32], side='left')
                  for b in range(B)])                               # [B, 31]

    # per (batch, global row): breakpoints, dedup by pixel keeping largest n
    rows = [[dict() for _ in range(NCORES * RPB)] for _ in range(B)]
    for b in range(B):
        for n in range(31):
            kk = int(k[b, n])
            if kk < S:
                rows[b][kk // W][kk % W] = n + 1   # knot index 1..31
    # m(b, s) = number of breakpoints with k <= s  -> segment index
    ks = [np.sort(k[b]) for b in range(B)]

    def seg(b, s):
        return int(np.searchsorted(ks[b], s, side='right'))

    n_left = n_right = 0
    per_row = []  # (p-major list per core) of (baseline, [(t,a,b) L], [(t,a,b) R])
    for c in range(NCORES):
        core_rows = []
        for b in range(B):
            for r in range(RPB):
                g = c * RPB + r
                s0 = c * PIX_PER_CORE + r * W
                bps = sorted(rows[b][g].items())   # [(col, knot_n)...]
                left = [(col, n) for col, n in bps if col < HALF]
                right = [(col, n) for col, n in bps if col >= HALF]
                n_left = max(n_left, len(left))
                n_right = max(n_right, len(right))
                mb = seg(b, s0 + HALF - 1)         # base line: valid at col HALF-1
                base = (a[b, mb], b2[b, mb])
                # left patches, latest-first: patch j covers cols < col_j with
                # the line of the segment *before* knot n_j
                lslots = []
                for col, n in sorted(left, reverse=True):
                    m_prev = seg(b, s0 + col - 1) if col > 0 else seg(b, s0 - 1)
                    lslots.append((x[b, n], a[b, m_prev], b2[b, m_prev]))
                # right patches, earliest-first: patch covers cols where
                # t >= x_n with the line of the segment *at* knot n
                rslots = []
                for col, n in sorted(right):
                    m_at = seg(b, s0 + col)
                    rslots.append((x[b, n], a[b, m_at], b2[b, m_at]))
                core_rows.append((base, lslots, rslots))
        per_row.append(core_rows)

    n_left = max(n_left, 1)
    n_right = max(n_right, 1)
    ncol = 2 + 3 * (n_left + n_right)
    params = []
    for c in range(NCORES):
        arr = np.zeros((P, ncol), dtype=np.float32)
        for p, (base, lslots, rslots) in enumerate(per_row[c]):
            arr[p, 0], arr[p, 1] = base
            col = 2
            for j in range(n_left):
                if j < len(lslots):
                    arr[p, col:col + 3] = lslots[j]
                else:
                    arr[p, col:col + 3] = (-1.0, 0.0, 0.0)   # mask t < -1 empty
                col += 3
            for j in range(n_right):
                if j < len(rslots):
                    arr[p, col:col + 3] = rslots[j]
                else:
                    arr[p, col:col + 3] = (2.0, 0.0, 0.0)    # mask t >= 2 empty
                col += 3
        params.append(arr)
    return params, (n_left, n_right)


def _build(n_left, n_right):
    import concourse.bacc as bacc
    import concourse.mybir as mybir
    from concourse.tile import TileContext

    f32 = mybir.dt.float32
    Alu = mybir.AluOpType
    ncol = 2 + 3 * (n_left + n_right)

    nc = bacc.Bacc("TRN2", debug=False)
    t_dram = nc.dram_tensor("t_tile", [P, W], f32, kind="ExternalInput").ap()
    prm_dram = nc.dram_tensor("params", [P, ncol], f32, kind="ExternalInput").ap()
    y_dram = nc.dram_tensor("y", [P, W], f32, kind="ExternalOutput").ap()

    with TileContext(nc) as tc:
        with tc.tile_pool(name="pool", bufs=1) as pool:
            prm = pool.tile([P, ncol], f32, tag="prm")
            nc.sync.dma_start(out=prm[:], in_=prm_dram[:])

            def sc(j):  # scalar AP = params column j
                return prm[:, j:j + 1]

            for ch in range(NCHUNK):
                c0, c1 = ch * CW, (ch + 1) * CW
                t = pool.tile([P, CW], f32, tag=f"t{ch}")
                o = pool.tile([P, CW], f32, tag=f"o{ch}")
                nc.sync.dma_start(out=t[:], in_=t_dram[:, c0:c1])
                # base line
                nc.vector.tensor_scalar(o[:], t[:], sc(0), sc(1), Alu.mult, Alu.add)
                # patches for this half
                if ch < NCHUNK // 2:
                    slots = [(2 + 3 * j, Alu.is_lt) for j in range(n_left)]
                else:
                    slots = [(2 + 3 * (n_left + j), Alu.is_ge) for j in range(n_right)]
                for base_col, cmp_op in slots:
                    m = pool.tile([P, CW], mybir.dt.uint8, tag=f"m{ch}")
                    ln = pool.tile([P, CW], f32, tag=f"l{ch}")
                    nc.vector.tensor_scalar(m[:], t[:], sc(base_col), None, cmp_op)
                    nc.vector.tensor_scalar(ln[:], t[:], sc(base_col + 1),
                                            sc(base_col + 2), Alu.mult, Alu.add)
                    nc.vector.copy_predicated(o[:], m[:], ln[:])
                nc.sync.dma_start(out=y_dram[:, c0:c1], in_=o[:])

    nc.compile()
    return nc


def _get_compiled(n_left, n_right):
    key = (n_left, n_right)
    if key not in _compiled:
        _compiled[key] = _build(n_left, n_right)
    return _compiled[key]


def kernel(segment_x, segment_y):
    from concourse.bass_utils import run_bass_kernel_spmd

    params, (n_left, n_right) = _host_prep(segment_x, segment_y)
    _, t_tiles = _get_t()
    nc = _get_compiled(n_left, n_right)
    in_maps = [{"t_tile": t_tiles[c], "params": params[c]} for c in range(NCORES)]
    res = run_bass_kernel_spmd(nc, in_maps, core_ids=list(range(NCORES)))

    out = np.empty((B, S), dtype=np.float32)
    for c in range(NCORES):
        yc = res.results[c]["y"]  # [128, 1536]
        base = c * PIX_PER_CORE
        out[:, base:base + PIX_PER_CORE] = yc.reshape(B, RPB * W)
    return out


# revision 3
# speedup vs baseline: 1.0196x; 1.0196x over previous
"""Trainium2 Bass kernel for nn_Decoder_58514634440787 (histogram_binning).

Piecewise-linear decoder: y[b, s] = interp of (segment_x, segment_y) knots
evaluated at the uniform pixel grid t_s = (s+1)/S, S = 196608, B = 8.

Strategy: the output along the pixel axis is piecewise linear with at most
33 knots per batch.  Pixels are sharded across 8 cores (24576 each) and laid
out on-chip as [128 partitions = 8 batches x 16 rows, 1536 pixels].  Each
1536-pixel row intersects at most a couple of knots, so per row the host
ships tiny line parameters (slope/intercept + knot thresholds) and the
device evaluates:

    out = base_a[p] * t + base_b[p]                       (one tensor_scalar)
    patch left  half where t <  tL[p] with line (aL, bL)  (mask + line + copy_predicated)
    patch right half where t >= tR[p] with line (aR, bR)

The mask compares are the exact same f32 predicates (t >= x_n) the reference
uses, so segment selection is bit-identical; line evaluation a*t+b matches
the reference's ratio*(x_in-x)+y to ~3e-4 elementwise worst case.

Inputs are the full [8, 33] knot tensors; sharding/gather happens here.
"""

import numpy as np

S = 196608
B = 8
W = 1536              # pixels per partition row
RPB = 16              # rows per batch per core
P = 128               # partitions = B * RPB
NCORES = 8
PIX_PER_CORE = RPB * W  # 24576
NCHUNK = 4
CW = W // NCHUNK      # 384 columns per chunk
HALF = W // 2

_t_grid = None          # f32 [S] exact (s+1)/S
_t_tiles = None         # per-core [128, 1536] f32
_compiled = {}          # (n_left, n_right) -> nc


def _get_t():
    global _t_grid, _t_tiles
    if _t_grid is None:
        _t_grid = (np.arange(1, S + 1, dtype=np.float64) / S).astype(np.float32)
        tiles = []
        for c in range(NCORES):
            base = c * PIX_PER_CORE
            rows = _t_grid[base:base + PIX_PER_CORE].reshape(RPB, W)
            tiles.append(np.tile(rows, (B, 1)))  # [128, 1536]
        _t_tiles = tiles
    return _t_grid, _t_tiles


def _fix_x_order(sx, sy):
    """Running max of x along the segment axis, y carried from the position
    achieving the max (ties keep the later entry). Matches reference."""
    x = sx.copy()
    y = sy.copy()
    for b in range(sx.shape[0]):
        cx, cy = sx[b, 0], sy[b, 0]
        for i in range(sx.shape[1]):
            if sx[b, i] >= cx:
                cx, cy = sx[b, i], sy[b, i]
            x[b, i] = cx
            y[b, i] = cy
    return x, y


def _host_prep(segment_x, segment_y):
    """Returns (params_per_core, (n_left, n_right)).

    params_per_core: list of [128, 2 + 3*(n_left + n_right)] f32 arrays with
    columns [base_a, base_b, (tL_j, aL_j, bL_j)..., (tR_j, aR_j, bR_j)...].
    Left slots are ordered latest-breakpoint-first (applied in order, each
    narrower than the previous); right slots earliest-first.
    """
    t_grid, _ = _get_t()
    sx = np.asarray(segment_x, dtype=np.float32)
    sy = np.asarray(segment_y, dtype=np.float32)
    x, y = _fix_x_order(sx, sy)

    gaps = x[:, 1:] - x[:, :-1]
    div = np.where(gaps == 0.0, np.float32(0.0001), gaps).astype(np.float32)
    a = ((y[:, 1:] - y[:, :-1]) / div).astype(np.float32)          # [B, 32]
    b2 = (y[:, :-1].astype(np.float64)
          - a.astype(np.float64) * x[:, :-1].astype(np.float64)).astype(np.float32)

    # First pixel index s with t_s >= x_n, for binning knots n = 1..31.
    # searchsorted on the exact f32 grid == the reference's f32 compares.
    k = np.stack([np.searchsorted(t_grid, x[b, 1:32], side='left')
                  for b in range(B)])                               # [B, 31]

    # per (batch, global row): breakpoints, dedup by pixel keeping largest n
    rows = [[dict() for _ in range(NCORES * RPB)] for _ in range(B)]
    for b in range(B):
        for n in range(31):
            kk = int(k[b, n])
            if kk < S:
                rows[b][kk // W][kk % W] = n + 1   # knot index 1..31
    # m(b, s) = number of breakpoints with k <= s  -> segment index
    ks = [np.sort(k[b]) for b in range(B)]

    def seg(b, s):
        return int(np.searchsorted(ks[b], s, side='right'))

    n_left = n_right = 0
    per_row = []  # (p-major list per core) of (baseline, [(t,a,b) L], [(t,a,b) R])
    for c in range(NCORES):
        core_rows = []
        for b in range(B):
            for r in range(RPB):
                g = c * RPB + r
                s0 = c * PIX_PER_CORE + r * W
                bps = sorted(rows[b][g].items())   # [(col, knot_n)...]
                left = [(col, n) for col, n in bps if col < HALF]
                right = [(col, n) for col, n in bps if col >= HALF]
                n_left = max(n_left, len(left))
                n_right = max(n_right, len(right))
                mb = seg(b, s0 + HALF - 1)         # base line: valid at col HALF-1
                base = (a[b, mb], b2[b, mb])
                # left patches, latest-first: patch j covers cols < col_j with
                # the line of the segment *before* knot n_j
                lslots = []
                for col, n in sorted(left, reverse=True):
                    m_prev = seg(b, s0 + col - 1) if col > 0 else seg(b, s0 - 1)
                    lslots.append((x[b, n], a[b, m_prev], b2[b, m_prev]))
                # right patches, earliest-first: patch covers cols where
                # t >= x_n with the line of the segment *at* knot n
                rslots = []
                for col, n in sorted(right):
                    m_at = seg(b, s0 + col)
                    rslots.append((x[b, n], a[b, m_at], b2[b, m_at]))
                core_rows.append((base, lslots, rslots))
        per_row.append(core_rows)

    n_left = max(n_left, 1)
    n_right = max(n_right, 1)
    ncol = 2 + 3 * (n_left + n_right)
    params = []
    for c in range(NCORES):
        arr = np.zeros((P, ncol), dtype=np.float32)
        for p, (base, lslots, rslots) in enumerate(per_row[c]):
            arr[p, 0], arr[p, 1] = base
            col = 2
            for j in range(n_left):
                if j < len(lslots):
                    arr[p, col:col + 3] = lslots[j]
                else:
                    arr[p, col:col + 3] = (-1.0, 0.0, 0.0)   # mask t < -1 empty
                col += 3
            for j in range(n_right):
                if j < len(rslots):
                    arr[p, col:col + 3] = rslots[j]
                else:
                    arr[p, col:col + 3] = (2.0, 0.0, 0.0)    # mask t >= 2 empty
                col += 3
        params.append(arr)
    return params, (n_left, n_right)


def _build(n_left, n_right):
    import concourse.bacc as bacc
    import concourse.mybir as mybir
    from concourse.tile import TileContext

    f32 = mybir.dt.float32
    Alu = mybir.AluOpType
    Act = mybir.ActivationFunctionType
    ncol = 2 + 3 * (n_left + n_right)

    nc = bacc.Bacc("TRN2", debug=False)
    t_dram = nc.dram_tensor("t_tile", [P, W], f32, kind="ExternalInput").ap()
    prm_dram = nc.dram_tensor("params", [P, ncol], f32, kind="ExternalInput").ap()
    y_dram = nc.dram_tensor("y", [P, W], f32, kind="ExternalOutput").ap()

    with TileContext(nc) as tc:
        with tc.tile_pool(name="pool", bufs=1) as pool:
            # params via the scalar-engine HWDGE queue so the sync queue can
            # start streaming t immediately
            prm = pool.tile([P, ncol], f32, tag="prm")
            nc.scalar.dma_start(out=prm[:], in_=prm_dram[:])

            def sc(j):  # scalar AP = params column j
                return prm[:, j:j + 1]

            for h in range(2):
                c0, c1 = h * HALF, (h + 1) * HALF
                t = pool.tile([P, HALF], f32, tag=f"t{h}")
                o = pool.tile([P, HALF], f32, tag=f"o{h}")
                nc.sync.dma_start(out=t[:], in_=t_dram[:, c0:c1])
                # base line on the Scalar engine: o = t*cA + cB
                nc.scalar.activation(o[:], t[:], Act.Identity,
                                     bias=sc(1), scale=sc(0))
                if h == 0:
                    slots = [(2 + 3 * j, Alu.is_lt) for j in range(n_left)]
                else:
                    slots = [(2 + 3 * (n_left + j), Alu.is_ge)
                             for j in range(n_right)]
                for base_col, cmp_op in slots:
                    m = pool.tile([P, HALF], mybir.dt.uint8, tag=f"m{h}")
                    ln = pool.tile([P, HALF], f32, tag=f"l{h}")
                    nc.vector.tensor_scalar(m[:], t[:], sc(base_col), None, cmp_op)
                    nc.scalar.activation(ln[:], t[:], Act.Identity,
                                         bias=sc(base_col + 2),
                                         scale=sc(base_col + 1))
                    nc.vector.copy_predicated(o[:], m[:], ln[:])
                nc.sync.dma_start(out=y_dram[:, c0:c1], in_=o[:])

    nc.compile()
    return nc


def _get_compiled(n_left, n_right):
    key = (n_left, n_right)
    if key not in _compiled:
        _compiled[key] = _build(n_left, n_right)
    return _compiled[key]


def kernel(segment_x, segment_y):
    from concourse.bass_utils import run_bass_kernel_spmd

    params, (n_left, n_right) = _host_prep(segment_x, segment_y)
    _, t_tiles = _get_t()
    nc = _get_compiled(n_left, n_right)
    in_maps = [{"t_tile": t_tiles[c], "params": params[c]} for c in range(NCORES)]
    res = run_bass_kernel_spmd(nc, in_maps, core_ids=list(range(NCORES)))

    out = np.empty((B, S), dtype=np.float32)
    for c in range(NCORES):
        yc = res.results[c]["y"]  # [128, 1536]
        base = c * PIX_PER_CORE
        out[:, base:base + PIX_PER_CORE] = yc.reshape(B, RPB * W)
    return out
